# revision 14
# baseline (speedup 1.0000x reference)
"""EdgeConv (ParticleNet-style) Trainium2 kernel, v3: gather-free.

Per event (16/core):
  - keys + diag mask on PE (fp32); top-16 via DVE max8/match_replace/max_index.
  - idx [128 p', 64 (4r+t)] u16 -> fp16 -> partition-fold DMA -> rep[0:1, 8192]
    (edge order n = p'*64 + 4r + t, i = 128t+p', slot k = r), then 7 doubling
    DMAs replicate to rep [128, 8192] fp16.
  - one-hot OH[jlo, jt, n] = (rep[jlo, n] == jlo + 128*jt), built per e-chunk
    on DVE (is_equal), GPSIMD (is_equal), ACT (Abs + Relu(1-|z|)).
  - gather-matmul per chunk: vgT_psum[h, n] = sum_jt v'_jt^T @ OH_jt  (PE),
    ACT copy -> bf16, DVE max with -p^T broadcast (relu(p+v) = p + max(v,-p)).
  - layer 2: out.T = sum_r (W2/16)^T @ m_r + (Wp@W2|b2hi|b2lo)^T @ [x;1;1].
"""

import numpy as np
import ml_dtypes


B, N, F = 128, 512, 32
K = 16
H, OUT = 128, 64
NCORES = 8
EV = B // NCORES
BIG = np.float32(1e30)
NE = K * N          # 8192 edges per event
CH = 2048           # e-chunk size
NCH = NE // CH      # 4 chunks
# one-hot builder per jt: engine assignment
OH_ENG = ["vector", "vector", "scalar", "scalar"]

_cache = {}


def _build_nc(n_ev=EV):
    import concourse.bass as bass
    import concourse.bacc as bacc
    import concourse.tile as tile
    import concourse.mybir as mybir
    from contextlib import ExitStack

    dt = mybir.dt
    AOT = mybir.AluOpType
    nc = bacc.Bacc("TRN2", target_bir_lowering=False, debug=False,
                   enable_asserts=False, num_devices=NCORES)

    xt_d = nc.dram_tensor("xt", [n_ev, F, N], dt.float32, kind="ExternalInput")
    wv_d = nc.dram_tensor("wv", [F + 1, H], dt.bfloat16, kind="ExternalInput")
    wpn_d = nc.dram_tensor("wpn", [F, H], dt.bfloat16, kind="ExternalInput")
    w2b_d = nc.dram_tensor("w2b", [H, OUT], dt.bfloat16, kind="ExternalInput")
    wx_d = nc.dram_tensor("wx", [F + 2, OUT], dt.bfloat16, kind="ExternalInput")
    diag_d = nc.dram_tensor("diag", [128, 4, N], dt.bfloat16, kind="ExternalInput")
    ident_d = nc.dram_tensor("ident", [128, 128], dt.bfloat16, kind="ExternalInput")
    iota_d = nc.dram_tensor("iota", [128, 4], dt.float16, kind="ExternalInput")
    iotb_d = nc.dram_tensor("iotb", [128, 4], dt.float32, kind="ExternalInput")
    out_d = nc.dram_tensor("out", [n_ev, OUT, N], dt.float32, kind="ExternalOutput")

    AF = mybir.ActivationFunctionType

    with tile.TileContext(nc) as tc, ExitStack() as ctx:
        cpool = ctx.enter_context(tc.tile_pool(name="consts", bufs=1))
        ident = cpool.tile([128, 128], dt.bfloat16)
        nc.sync.dma_start(ident[:], ident_d[:])
        diag = cpool.tile([128, 4, N], dt.bfloat16)
        nc.sync.dma_start(diag[:], diag_d[:])
        wv = cpool.tile([F + 1, H], dt.bfloat16)
        nc.sync.dma_start(wv[:], wv_d[:])
        wpn = cpool.tile([F, H], dt.bfloat16)
        nc.sync.dma_start(wpn[:], wpn_d[:])
        w2b = cpool.tile([H, OUT], dt.bfloat16)
        nc.sync.dma_start(w2b[:], w2b_d[:])
        wx = cpool.tile([F + 2, OUT], dt.bfloat16)
        nc.sync.dma_start(wx[:], wx_d[:])
        iota = cpool.tile([128, 4], dt.float16)
        nc.sync.dma_start(iota[:], iota_d[:])
        iotb = cpool.tile([128, 4], dt.float32)
        nc.sync.dma_start(iotb[:], iotb_d[:])

        l4s = [cpool.tile([4, N], dt.float32, name=f"l4c{q}") for q in range(2)]
        xtbs = [cpool.tile([F + 2, N], dt.bfloat16, name=f"xtbc{q}")
                for q in range(2)]
        for q in range(2):
            nc.gpsimd.memset(l4s[q][0:2, :], -1.0)
            nc.gpsimd.memset(xtbs[q][F:F + 2, :], 1.0)

        xt_pool = ctx.enter_context(tc.tile_pool(name="xt", bufs=2))
        r4_pool = ctx.enter_context(tc.tile_pool(name="r4", bufs=2))
        keys_pool = ctx.enter_context(tc.tile_pool(name="keys", bufs=2))
        keys2_pool = ctx.enter_context(tc.tile_pool(name="keys2", bufs=2))
        vals_pool = ctx.enter_context(tc.tile_pool(name="vals", bufs=2))
        idxs_pool = ctx.enter_context(tc.tile_pool(name="idxs", bufs=2))
        idxf_pool = ctx.enter_context(tc.tile_pool(name="idxf", bufs=2))
        rep_pool = ctx.enter_context(tc.tile_pool(name="rep", bufs=2))
        oh_pool = ctx.enter_context(tc.tile_pool(name="oh", bufs=2))
        t1_pool = ctx.enter_context(tc.tile_pool(name="t1", bufs=2))
        vsb_pool = ctx.enter_context(tc.tile_pool(name="vsb", bufs=2))
        pn_pool = ctx.enter_context(tc.tile_pool(name="pn", bufs=2))
        vgT_pool = ctx.enter_context(tc.tile_pool(name="vgT", bufs=2))
        mb8_pool = ctx.enter_context(tc.tile_pool(name="mb8", bufs=2))
        mb4_pool = ctx.enter_context(tc.tile_pool(name="mb4", bufs=2))
        mb2_pool = ctx.enter_context(tc.tile_pool(name="mb2", bufs=2))
        mbar_pool = ctx.enter_context(tc.tile_pool(name="mbar", bufs=2))
        outsb_pool = ctx.enter_context(tc.tile_pool(name="outsb", bufs=2))

        kps_pool = ctx.enter_context(tc.tile_pool(name="kps", bufs=1, space="PSUM"))
        vps_pool = ctx.enter_context(tc.tile_pool(name="vps", bufs=1, space="PSUM"))
        pnp_pool = ctx.enter_context(tc.tile_pool(name="pnp", bufs=1, space="PSUM"))
        ops_pool = ctx.enter_context(tc.tile_pool(name="ops", bufs=1, space="PSUM"))
        gps_pool = ctx.enter_context(tc.tile_pool(name="gps", bufs=1, space="PSUM"))

        for e in range(n_ev):
            xt = xt_pool.tile([F, N], dt.float32)
            nc.sync.dma_start(xt[:], xt_d[e])
            l4 = l4s[e % 2]
            nc.sync.dma_start(l4[2:4, :], xt_d[e][0:2, :])
            r4 = r4_pool.tile([4, N], dt.float32)
            nc.sync.dma_start(r4[2:4, :], xt_d[e][0:2, :])
            nc.scalar.activation(r4[0:2, :], xt[0:2, :], AF.Square,
                                 scale=float(np.sqrt(0.5)))
            xtb = xtbs[e % 2]
            nc.scalar.activation(xtb[0:F, :], xt[:], AF.Copy)

            # ---- v' node-major slabs [p=j%128, jt, h]
            vps = vps_pool.tile([128, N], dt.float32)
            for c in range(4):
                nc.tensor.matmul(vps[:, H * c:H * (c + 1)],
                                 xtb[0:F + 1, 128 * c:128 * (c + 1)],
                                 wv[:], start=True, stop=True)
            v_sb = vsb_pool.tile([128, 4, H], dt.bfloat16)
            nc.scalar.activation(v_sb[:].opt(), vps[:], AF.Copy)

            # ---- -p^T in (p', t) column order
            pnp = pnp_pool.tile([128, N], dt.float32)
            nc.tensor.matmul(pnp[:], wpn[:],
                             xtb[0:F, :].rearrange("f (t p) -> f p t",
                                                   t=4, p=128),
                             start=True, stop=True)
            pn_sb = pn_pool.tile([H, N], dt.bfloat16)
            nc.scalar.activation(pn_sb[:], pnp[:], AF.Copy)

            # ---- keys + selection
            keys = keys_pool.tile([128, 4, N], dt.float32)
            keys2 = keys2_pool.tile([128, 4, N], dt.float32)
            vals = vals_pool.tile([128, 64], dt.float32)
            idxs = idxs_pool.tile([128, 64], dt.uint16)
            for t in range(4):
                kps = kps_pool.tile([128, N], dt.float32)
                nc.tensor.matmul(kps[:], l4[:, 128 * t:128 * (t + 1)], r4[:],
                                 start=True, stop=False)
                nc.tensor.matmul(kps[:], ident[:], diag[:, t, :],
                                 start=False, stop=True)
                kt = keys[:, t, :].opt()
                k2t = keys2[:, t, :].opt()
                nc.scalar.activation(kt, kps[:], AF.Copy)
                v0 = vals[:, 16 * t:16 * t + 8]
                v1 = vals[:, 16 * t + 8:16 * t + 16]
                idxs4 = idxs[:].rearrange("p (r t) -> p r t", r=16, t=4)
                i0 = idxs4[:, 0:8, t].opt()
                i1 = idxs4[:, 8:16, t].opt()
                nc.vector.max(v0, kt)
                nc.vector.match_replace(k2t, v0, kt, -float(BIG))
                nc.vector.max(v1, k2t)
                nc.vector.max_index(i0, v0, kt)
                nc.vector.max_index(i1, v1, k2t)

            # ---- idx -> fp16 -> flat [1, 8192] -> replicate [128, 8192]
            idxf = idxf_pool.tile([128, 64], dt.float16)
            nc.vector.tensor_copy(idxf[:], idxs[:])
            rep = rep_pool.tile([128, NE], dt.float16)
            nc.sync.dma_start(
                rep[0:1, :].rearrange("q (p c) -> q p c", p=128, c=64),
                idxf[:])
            w = 1
            while w < 128:
                nc.sync.dma_start(rep[w:2 * w, :], rep[0:w, :])
                w *= 2

            # ---- chunked one-hot + gather-matmul + max
            vgT = vgT_pool.tile([128, NE], dt.bfloat16)
            for c in range(NCH):
                sl = slice(CH * c, CH * (c + 1))
                oh = oh_pool.tile([128, 4, CH], dt.bfloat16)
                for jt in range(4):
                    eng = OH_ENG[jt]
                    if eng == "scalar":
                        t1 = t1_pool.tile([128, CH], dt.float16)
                        nc.scalar.activation(t1[:], rep[:, sl], AF.Abs,
                                             bias=iotb[:, jt:jt + 1])
                        nc.scalar.activation(oh[:, jt, :].opt(), t1[:],
                                             AF.Relu, scale=-1.0, bias=1.0)
                    else:
                        nc.vector.tensor_scalar(
                            oh[:, jt, :].opt(), rep[:, sl],
                            iotb[:, jt:jt + 1], 0.0,
                            op0=AOT.add, op1=AOT.is_equal)
                gps = gps_pool.tile([128, CH], dt.float32)
                for jt in range(4):
                    for s in range(CH // 512):
                        nc.tensor.matmul(
                            gps[:, 512 * s:512 * (s + 1)],
                            v_sb[:, jt, :],
                            oh[:, jt, 512 * s:512 * (s + 1)].opt(),
                            start=(jt == 0), stop=(jt == 3))
                nc.scalar.activation(vgT[:, sl], gps[:], AF.Copy)
            # m = max(v_j, -p_i): n = p'*64 + 4r + t; i = 128t + p'
            vg4 = vgT[:].rearrange("h (p r t) -> h p r t", p=128, r=K, t=4)
            pnb = pn_sb[:].rearrange("h (p a t) -> h p a t",
                                     p=128, a=1, t=4).broadcast_to(
                [128, 128, K, 4])
            nc.vector.tensor_tensor(vg4, vg4, pnb, op=AOT.max)

            # ---- r-sum tree: mbar[h, (p', t)] = sum_r m_r, then 2 matmuls
            mb8 = mb8_pool.tile([128, 128, 8, 4], dt.float16)
            nc.vector.tensor_tensor(mb8[:], vg4[:, :, 0:8, :],
                                    vg4[:, :, 8:16, :], op=AOT.add)
            mb4 = mb4_pool.tile([128, 128, 4, 4], dt.float16)
            nc.gpsimd.tensor_tensor(mb4[:], mb8[:, :, 0:4, :],
                                    mb8[:, :, 4:8, :], op=AOT.add)
            mb2 = mb2_pool.tile([128, 128, 2, 4], dt.float16)
            nc.gpsimd.tensor_tensor(mb2[:], mb4[:, :, 0:2, :],
                                    mb4[:, :, 2:4, :], op=AOT.add)
            mbar = mbar_pool.tile([128, 128, 4], dt.bfloat16)
            nc.gpsimd.tensor_tensor(mbar[:], mb2[:, :, 0, :],
                                    mb2[:, :, 1, :], op=AOT.add)

            ops = ops_pool.tile([OUT, N], dt.float32)
            nc.tensor.matmul(ops[:], w2b[:], mbar[:].opt(),
                             start=True, stop=False)
            nc.tensor.matmul(
                ops[:], wx[:],
                xtb[:].rearrange("f (t p) -> f p t", t=4, p=128),
                start=False, stop=True)
            # ops cols are (p', t); scatter into osb natural order i = 128t+p'
            osb = outsb_pool.tile([OUT, N], dt.float32)
            nc.scalar.activation(
                osb[:].rearrange("o (t p) -> o p t", t=4, p=128),
                ops[:].rearrange("o (p t) -> o p t", p=128, t=4), AF.Copy)
            nc.sync.dma_start(out_d[e], osb[:])

    nc.compile()
    return nc


def _prep_inputs(x, W1, b1, W2, b2):
    bf16 = ml_dtypes.bfloat16
    Wp = (W1[0:F, :].astype(np.float64) - W1[F:2 * F, :].astype(np.float64))
    Wv = W1[F:2 * F, :].astype(np.float32)

    wv = np.zeros((F + 1, H), dtype=bf16)
    wv[0:F, :] = Wv.astype(bf16)
    wv[F, :] = b1.astype(bf16)

    wpn = (-Wp).astype(bf16)
    w2b = (W2.astype(np.float32) / np.float32(K)).astype(bf16)

    Wpp = (Wp @ W2.astype(np.float64)).astype(bf16)
    b2f = b2.astype(np.float32)
    b2hi = b2f.astype(bf16)
    b2lo = (b2f - b2hi.astype(np.float32)).astype(bf16)
    wx = np.zeros((F + 2, OUT), dtype=bf16)
    wx[0:F, :] = Wpp
    wx[F, :] = b2hi
    wx[F + 1, :] = b2lo

    diag = np.zeros((128, 4, N), dtype=bf16)
    for t in range(4):
        diag[np.arange(128), t, t * 128 + np.arange(128)] = bf16(-BIG)
    ident = np.eye(128, dtype=bf16)

    p128 = np.arange(128, dtype=np.float32)[:, None]
    jt4 = np.arange(4, dtype=np.float32)[None, :]
    iota = (p128 + 128 * jt4).astype(np.float16)
    iotb = -(p128 + 128 * jt4).astype(np.float32)

    xt = np.ascontiguousarray(x.transpose(0, 2, 1).astype(np.float32))
    return xt, wv, wpn, w2b, wx, diag, ident, iota, iotb


def _in_maps(xt, wv, wpn, w2b, wx, diag, ident, iota, iotb):
    return [{
        "xt": xt[c * EV:(c + 1) * EV],
        "wv": wv, "wpn": wpn, "w2b": w2b, "wx": wx,
        "diag": diag, "ident": ident, "iota": iota, "iotb": iotb,
    } for c in range(NCORES)]


def kernel(x, W1, b1, W2, b2):
    from concourse.bass_utils import run_bass_kernel_spmd

    key = "nc"
    if key not in _cache:
        _cache[key] = _build_nc()
    nc = _cache[key]

    prepped = _prep_inputs(
        np.asarray(x), np.asarray(W1), np.asarray(b1),
        np.asarray(W2), np.asarray(b2))

    res = run_bass_kernel_spmd(nc, _in_maps(*prepped), list(range(NCORES)))
    outs = [res.results[c]["out"] for c in range(NCORES)]
    full = np.concatenate(outs, axis=0)
    return np.ascontiguousarray(full.transpose(0, 2, 1)).astype(np.float32)


# revision 15
# speedup vs baseline: 1.0276x; 1.0276x over previous
"""EdgeConv (ParticleNet-style) Trainium2 kernel, v3: gather-free.

Per event (16/core):
  - keys + diag mask on PE (fp32); top-16 via DVE max8/match_replace/max_index.
  - idx [128 p', 64 (4r+t)] u16 -> fp16 -> partition-fold DMA -> rep[0:1, 8192]
    (edge order n = p'*64 + 4r + t, i = 128t+p', slot k = r), then 7 doubling
    DMAs replicate to rep [128, 8192] fp16.
  - one-hot OH[jlo, jt, n] = (rep[jlo, n] == jlo + 128*jt), built per e-chunk
    on DVE (is_equal), GPSIMD (is_equal), ACT (Abs + Relu(1-|z|)).
  - gather-matmul per chunk: vgT_psum[h, n] = sum_jt v'_jt^T @ OH_jt  (PE),
    ACT copy -> bf16, DVE max with -p^T broadcast (relu(p+v) = p + max(v,-p)).
  - layer 2: out.T = sum_r (W2/16)^T @ m_r + (Wp@W2|b2hi|b2lo)^T @ [x;1;1].
"""

import numpy as np
import ml_dtypes

B, N, F = 128, 512, 32
K = 16
H, OUT = 128, 64
NCORES = 8
EV = B // NCORES
BIG = np.float32(1e30)
NE = K * N          # 8192 edges per event
CH = 2048           # e-chunk size
NCH = NE // CH      # 4 chunks
# one-hot builder per jt: engine assignment
OH_ENG = ["vector", "vector", "scalar", "scalar"]

_cache = {}


def _build_nc(n_ev=EV):
    import concourse.bass as bass
    import concourse.bacc as bacc
    import concourse.tile as tile
    import concourse.mybir as mybir
    from contextlib import ExitStack

    dt = mybir.dt
    AOT = mybir.AluOpType
    nc = bacc.Bacc("TRN2", target_bir_lowering=False, debug=False,
                   enable_asserts=False, num_devices=NCORES)

    xt_d = nc.dram_tensor("xt", [n_ev, F, N], dt.float32, kind="ExternalInput")
    wv_d = nc.dram_tensor("wv", [F + 1, H], dt.bfloat16, kind="ExternalInput")
    wpn_d = nc.dram_tensor("wpn", [F, H], dt.bfloat16, kind="ExternalInput")
    w2b_d = nc.dram_tensor("w2b", [H, OUT], dt.bfloat16, kind="ExternalInput")
    wx_d = nc.dram_tensor("wx", [F + 2, OUT], dt.bfloat16, kind="ExternalInput")
    diag_d = nc.dram_tensor("diag", [128, 4, N], dt.bfloat16, kind="ExternalInput")
    ident_d = nc.dram_tensor("ident", [128, 128], dt.bfloat16, kind="ExternalInput")
    iota_d = nc.dram_tensor("iota", [128, 4], dt.float16, kind="ExternalInput")
    iotb_d = nc.dram_tensor("iotb", [128, 4], dt.float32, kind="ExternalInput")
    out_d = nc.dram_tensor("out", [n_ev, OUT, N], dt.float32, kind="ExternalOutput")

    AF = mybir.ActivationFunctionType

    with tile.TileContext(nc) as tc, ExitStack() as ctx:
        cpool = ctx.enter_context(tc.tile_pool(name="consts", bufs=1))
        ident = cpool.tile([128, 128], dt.bfloat16)
        nc.sync.dma_start(ident[:], ident_d[:])
        diag = cpool.tile([128, 4, N], dt.bfloat16)
        nc.sync.dma_start(diag[:], diag_d[:])
        wv = cpool.tile([F + 1, H], dt.bfloat16)
        nc.sync.dma_start(wv[:], wv_d[:])
        wpn = cpool.tile([F, H], dt.bfloat16)
        nc.sync.dma_start(wpn[:], wpn_d[:])
        w2b = cpool.tile([H, OUT], dt.bfloat16)
        nc.sync.dma_start(w2b[:], w2b_d[:])
        wx = cpool.tile([F + 2, OUT], dt.bfloat16)
        nc.sync.dma_start(wx[:], wx_d[:])
        iota = cpool.tile([128, 4], dt.float16)
        nc.sync.dma_start(iota[:], iota_d[:])
        iotb = cpool.tile([128, 4], dt.float32)
        nc.sync.dma_start(iotb[:], iotb_d[:])

        l4s = [cpool.tile([4, N], dt.float32, name=f"l4c{q}") for q in range(2)]
        xtbs = [cpool.tile([F + 2, N], dt.bfloat16, name=f"xtbc{q}")
                for q in range(2)]
        for q in range(2):
            nc.gpsimd.memset(l4s[q][0:2, :], -1.0)
            nc.gpsimd.memset(xtbs[q][F:F + 2, :], 1.0)

        xt_pool = ctx.enter_context(tc.tile_pool(name="xt", bufs=2))
        r4_pool = ctx.enter_context(tc.tile_pool(name="r4", bufs=2))
        keys_pool = ctx.enter_context(tc.tile_pool(name="keys", bufs=2))
        keys2_pool = ctx.enter_context(tc.tile_pool(name="keys2", bufs=2))
        vals_pool = ctx.enter_context(tc.tile_pool(name="vals", bufs=2))
        idxs_pool = ctx.enter_context(tc.tile_pool(name="idxs", bufs=2))
        idxf_pool = ctx.enter_context(tc.tile_pool(name="idxf", bufs=2))
        rep_pool = ctx.enter_context(tc.tile_pool(name="rep", bufs=2))
        oh_pool = ctx.enter_context(tc.tile_pool(name="oh", bufs=2))
        t1_pool = ctx.enter_context(tc.tile_pool(name="t1", bufs=2))
        vsb_pool = ctx.enter_context(tc.tile_pool(name="vsb", bufs=2))
        pn_pool = ctx.enter_context(tc.tile_pool(name="pn", bufs=2))
        vgT_pool = ctx.enter_context(tc.tile_pool(name="vgT", bufs=2))
        outsb_pool = ctx.enter_context(tc.tile_pool(name="outsb", bufs=2))

        kps_pool = ctx.enter_context(tc.tile_pool(name="kps", bufs=1, space="PSUM"))
        vps_pool = ctx.enter_context(tc.tile_pool(name="vps", bufs=1, space="PSUM"))
        pnp_pool = ctx.enter_context(tc.tile_pool(name="pnp", bufs=1, space="PSUM"))
        ops_pool = ctx.enter_context(tc.tile_pool(name="ops", bufs=1, space="PSUM"))
        gps_pool = ctx.enter_context(tc.tile_pool(name="gps", bufs=1, space="PSUM"))

        for e in range(n_ev):
            xt = xt_pool.tile([F, N], dt.float32)
            nc.sync.dma_start(xt[:], xt_d[e])
            l4 = l4s[e % 2]
            nc.sync.dma_start(l4[2:4, :], xt_d[e][0:2, :])
            r4 = r4_pool.tile([4, N], dt.float32)
            nc.sync.dma_start(r4[2:4, :], xt_d[e][0:2, :])
            nc.scalar.activation(r4[0:2, :], xt[0:2, :], AF.Square,
                                 scale=float(np.sqrt(0.5)))
            xtb = xtbs[e % 2]
            nc.scalar.activation(xtb[0:F, :], xt[:], AF.Copy)

            # ---- v' node-major slabs [p=j%128, jt, h]
            vps = vps_pool.tile([128, N], dt.float32)
            for c in range(4):
                nc.tensor.matmul(vps[:, H * c:H * (c + 1)],
                                 xtb[0:F + 1, 128 * c:128 * (c + 1)],
                                 wv[:], start=True, stop=True)
            v_sb = vsb_pool.tile([128, 4, H], dt.bfloat16)
            nc.scalar.activation(v_sb[:].opt(), vps[:], AF.Copy)

            # ---- -p^T in (p', t) column order
            pnp = pnp_pool.tile([128, N], dt.float32)
            nc.tensor.matmul(pnp[:], wpn[:],
                             xtb[0:F, :].rearrange("f (t p) -> f p t",
                                                   t=4, p=128),
                             start=True, stop=True)
            pn_sb = pn_pool.tile([H, N], dt.bfloat16)
            nc.scalar.activation(pn_sb[:], pnp[:], AF.Copy)

            # ---- keys + selection
            keys = keys_pool.tile([128, 4, N], dt.float32)
            keys2 = keys2_pool.tile([128, 4, N], dt.float32)
            vals = vals_pool.tile([128, 64], dt.float32)
            idxs = idxs_pool.tile([128, 64], dt.uint16)
            for t in range(4):
                kps = kps_pool.tile([128, N], dt.float32)
                nc.tensor.matmul(kps[:], l4[:, 128 * t:128 * (t + 1)], r4[:],
                                 start=True, stop=False)
                nc.tensor.matmul(kps[:], ident[:], diag[:, t, :],
                                 start=False, stop=True)
                kt = keys[:, t, :].opt()
                k2t = keys2[:, t, :].opt()
                nc.scalar.activation(kt, kps[:], AF.Copy)
                v0 = vals[:, 16 * t:16 * t + 8]
                v1 = vals[:, 16 * t + 8:16 * t + 16]
                idxs4 = idxs[:].rearrange("p (r t) -> p r t", r=16, t=4)
                i0 = idxs4[:, 0:8, t].opt()
                i1 = idxs4[:, 8:16, t].opt()
                nc.vector.max(v0, kt)
                nc.vector.match_replace(k2t, v0, kt, -float(BIG))
                nc.vector.max(v1, k2t)
                nc.vector.max_index(i0, v0, kt)
                nc.vector.max_index(i1, v1, k2t)

            # ---- idx -> fp16 -> flat [1, 8192] -> replicate [128, 8192]
            idxf = idxf_pool.tile([128, 64], dt.float16)
            nc.vector.tensor_copy(idxf[:], idxs[:])
            rep = rep_pool.tile([128, NE], dt.float16)
            nc.sync.dma_start(
                rep[0:1, :].rearrange("q (p c) -> q p c", p=128, c=64),
                idxf[:])
            w = 1
            while w < 128:
                nc.sync.dma_start(rep[w:2 * w, :], rep[0:w, :])
                w *= 2

            # ---- chunked one-hot + gather-matmul + max
            vgT = vgT_pool.tile([128, NE], dt.bfloat16)
            for c in range(NCH):
                sl = slice(CH * c, CH * (c + 1))
                oh = oh_pool.tile([128, 4, CH], dt.bfloat16)
                for jt in range(4):
                    eng = OH_ENG[jt]
                    if eng == "scalar":
                        t1 = t1_pool.tile([128, CH], dt.float16)
                        nc.scalar.activation(t1[:], rep[:, sl], AF.Abs,
                                             bias=iotb[:, jt:jt + 1])
                        nc.scalar.activation(oh[:, jt, :].opt(), t1[:],
                                             AF.Relu, scale=-1.0, bias=1.0)
                    else:
                        iob = iota[:, jt:jt + 1].rearrange(
                            "p a -> p a").broadcast_to([128, CH])
                        engine = nc.vector if eng == "vector" else nc.gpsimd
                        engine.tensor_tensor(oh[:, jt, :].opt(), rep[:, sl],
                                             iob, op=AOT.is_equal)
                gps = gps_pool.tile([128, CH], dt.float32)
                for jt in range(4):
                    for s in range(CH // 512):
                        nc.tensor.matmul(
                            gps[:, 512 * s:512 * (s + 1)],
                            v_sb[:, jt, :],
                            oh[:, jt, 512 * s:512 * (s + 1)].opt(),
                            start=(jt == 0), stop=(jt == 3))
                nc.scalar.activation(vgT[:, sl], gps[:], AF.Copy)
            # m = max(v_j, -p_i): n = p'*64 + 4r + t; i = 128t + p'
            vg4 = vgT[:].rearrange("h (p r t) -> h p r t", p=128, r=K, t=4)
            pnb = pn_sb[:].rearrange("h (p a t) -> h p a t",
                                     p=128, a=1, t=4).broadcast_to(
                [128, 128, K, 4])
            nc.vector.tensor_tensor(vg4, vg4, pnb, op=AOT.max)

            # ---- layer 2 over r-slices
            ops = ops_pool.tile([OUT, N], dt.float32)
            for r in range(K):
                rhs = vgT[:].rearrange("h (p r t) -> h r p t",
                                       p=128, r=K, t=4)[:, r].opt()
                nc.tensor.matmul(ops[:], w2b[:], rhs,
                                 start=(r == 0), stop=False)
            nc.tensor.matmul(
                ops[:], wx[:],
                xtb[:].rearrange("f (t p) -> f p t", t=4, p=128),
                start=False, stop=True)
            # ops cols are (p', t); scatter into osb natural order i = 128t+p'
            osb = outsb_pool.tile([OUT, N], dt.float32)
            nc.scalar.activation(
                osb[:].rearrange("o (t p) -> o p t", t=4, p=128),
                ops[:].rearrange("o (p t) -> o p t", p=128, t=4), AF.Copy)
            nc.sync.dma_start(out_d[e], osb[:])

    nc.compile()
    return nc


def _prep_inputs(x, W1, b1, W2, b2):
    bf16 = ml_dtypes.bfloat16
    Wp = (W1[0:F, :].astype(np.float64) - W1[F:2 * F, :].astype(np.float64))
    Wv = W1[F:2 * F, :].astype(np.float32)

    wv = np.zeros((F + 1, H), dtype=bf16)
    wv[0:F, :] = Wv.astype(bf16)
    wv[F, :] = b1.astype(bf16)

    wpn = (-Wp).astype(bf16)
    w2b = (W2.astype(np.float32) / np.float32(K)).astype(bf16)

    Wpp = (Wp @ W2.astype(np.float64)).astype(bf16)
    b2f = b2.astype(np.float32)
    b2hi = b2f.astype(bf16)
    b2lo = (b2f - b2hi.astype(np.float32)).astype(bf16)
    wx = np.zeros((F + 2, OUT), dtype=bf16)
    wx[0:F, :] = Wpp
    wx[F, :] = b2hi
    wx[F + 1, :] = b2lo

    diag = np.zeros((128, 4, N), dtype=bf16)
    for t in range(4):
        diag[np.arange(128), t, t * 128 + np.arange(128)] = bf16(-BIG)
    ident = np.eye(128, dtype=bf16)

    p128 = np.arange(128, dtype=np.float32)[:, None]
    jt4 = np.arange(4, dtype=np.float32)[None, :]
    iota = (p128 + 128 * jt4).astype(np.float16)
    iotb = -(p128 + 128 * jt4).astype(np.float32)

    xt = np.ascontiguousarray(x.transpose(0, 2, 1).astype(np.float32))
    return xt, wv, wpn, w2b, wx, diag, ident, iota, iotb


def _in_maps(xt, wv, wpn, w2b, wx, diag, ident, iota, iotb):
    return [{
        "xt": xt[c * EV:(c + 1) * EV],
        "wv": wv, "wpn": wpn, "w2b": w2b, "wx": wx,
        "diag": diag, "ident": ident, "iota": iota, "iotb": iotb,
    } for c in range(NCORES)]


def kernel(x, W1, b1, W2, b2):
    from concourse.bass_utils import run_bass_kernel_spmd

    key = "nc"
    if key not in _cache:
        _cache[key] = _build_nc()
    nc = _cache[key]

    prepped = _prep_inputs(
        np.asarray(x), np.asarray(W1), np.asarray(b1),
        np.asarray(W2), np.asarray(b2))

    res = run_bass_kernel_spmd(nc, _in_maps(*prepped), list(range(NCORES)))
    outs = [res.results[c]["out"] for c in range(NCORES)]
    full = np.concatenate(outs, axis=0)
    return np.ascontiguousarray(full.transpose(0, 2, 1)).astype(np.float32)
